# revision 1
# baseline (speedup 1.0000x reference)
"""Biaffine span classifier kernel for 8 Trainium2 NeuronCores.

Math (per batch b, label o):
    start = relu(x @ W_start + b_start); end = relu(x @ W_end + b_end)
    rotate both with tiled-halves sinusoidal tables
    span[o,x,y] = startR[x,:] @ weight[o] @ endR[y,:]^T
    span = span*pad[y] - (1-pad[y])*NEG - NEG*tril(x>y)

Sharding: core c = b*2 + half handles batch b and labels [half*8, half*8+8).
Each core writes a contiguous [8, S, S] slice of the output.

On-chip layout is transposed ([H, S], H on partitions); x is transposed on
the host so every contraction has its reduction dim on partitions. All
matmuls run in fp32r mode (single-pass PE, ~TF32 precision) — operands are
rounded to fp32r by their producers as the BIR verifier requires. Start and
end projections share one matmul chain (stacked [W_start|W_end] stationary
operand); the end half is moved to partitions 0-63 by selector matmuls that
also produce the rotation's pair-swapped values. The mask's additive term is
folded into the big matmul via an augmented K=65 contraction (ones row in
tmpT, add_row in endT). Blocks entirely below the diagonal are exactly -NEG
in fp32 (|span| << 0.5*ulp(NEG)); their output regions are written once
during prep from a constant band on the SWDGE queue, hiding that DMA under
setup compute. Each label's remaining output goes out as a 2 MB contiguous
chunk (rows 0-511) plus a 1 MB strided chunk (rows 512-1023, y >= 512),
double-buffered so DMA, PE, DVE and ACT overlap.
"""

import numpy as np

B, S, I, H, O = 4, 1024, 768, 64, 16
NCORES = 8
OH = O // 2  # 8 labels per core
NEG = 1.0e12
KT = I // 128  # 6 k-tiles over the input dim
ST = S // 128  # 8 s-tiles

_STATE = {}


def _tables():
    """Host-precomputed constants (mimic reference fp32 ops)."""
    position = np.arange(S, dtype=np.float32)
    idx = np.arange(H // 2, dtype=np.float32)
    expo = (np.float32(-2.0) * idx) / np.float32(H)
    inv_freq = np.power(np.float32(10000.0), expo).astype(np.float32)
    ang = position[:, None] * inv_freq[None, :]          # [S, 32] f32
    cos_h = np.cos(ang).astype(np.float32).T             # [32, S]
    sin_h = np.sin(ang).astype(np.float32).T
    cosT = np.ascontiguousarray(np.concatenate([cos_h, cos_h], axis=0))  # [64, S]
    sinT = np.ascontiguousarray(np.concatenate([sin_h, sin_h], axis=0))
    # pair-swap as lhsT: out[2m] = -in[2m+1]; out[2m+1] = in[2m]
    msw = np.zeros((H, H), np.float32)
    for m in range(H // 2):
        msw[2 * m + 1, 2 * m] = -1.0
        msw[2 * m, 2 * m + 1] = 1.0
    # selectors on the stacked [start; end] projection (lhsT, [128, 192]):
    # [:, 0:64] swap start rows; [:, 64:128] extract end rows; [:, 128:192]
    # swap end rows
    sel = np.zeros((2 * H, 3 * H), np.float32)
    sel[0:H, 0:H] = msw
    sel[H:2 * H, H:2 * H] = np.eye(H, dtype=np.float32)
    sel[H:2 * H, 2 * H:3 * H] = msw
    # compressed tril window: T[x', v] = -NEG if x'+384 > v; pattern k for
    # diagonal-crossing blocks is the slice [:, 384-128k : 896-128k]
    xp = np.arange(128, dtype=np.int64)[:, None]
    vp = np.arange(896, dtype=np.int64)[None, :]
    tril = np.where(xp + 384 > vp, np.float32(-NEG),
                    np.float32(0.0)).astype(np.float32)   # [128, 896]
    return cosT, sinT, sel, tril


def _build():
    import concourse.bacc as bacc
    import concourse.bass as bass
    import concourse.mybir as mybir
    from concourse import tile

    f32 = mybir.dt.float32
    f32r = mybir.dt.float32r
    AF = mybir.ActivationFunctionType
    ALU = mybir.AluOpType
    PSUM = bass.MemorySpace.PSUM

    nc = bacc.Bacc("TRN2", target_bir_lowering=False, debug=False,
                   num_devices=NCORES)

    xT_t = nc.dram_tensor("xT", [I, S], f32, kind="ExternalInput")
    mask_t = nc.dram_tensor("mask", [1, S], f32, kind="ExternalInput")
    wb_t = nc.dram_tensor("w_both", [I, 2 * H], f32, kind="ExternalInput")
    b2_t = nc.dram_tensor("bias2", [2 * H, 1], f32, kind="ExternalInput")
    wo_t = nc.dram_tensor("w_o", [OH, H, H], f32, kind="ExternalInput")
    cos_t = nc.dram_tensor("cos_t", [H, S], f32, kind="ExternalInput")
    sin_t = nc.dram_tensor("sin_t", [H, S], f32, kind="ExternalInput")
    sel_t = nc.dram_tensor("sel3", [2 * H, 3 * H], f32, kind="ExternalInput")
    tril_t = nc.dram_tensor("trilneg", [128, 896], f32, kind="ExternalInput")
    out_t = nc.dram_tensor("out", [OH, S, S], f32, kind="ExternalOutput")

    # [o, c, p, xb, y]: row = 512c + 128xb + p
    out_r = out_t.ap().rearrange("o (c xb p) y -> o c p xb y", c=2, xb=4, p=128)

    def r(ap):
        return ap.bitcast(f32r)

    with tile.TileContext(nc) as tc:
        with tc.tile_pool(name="persist", bufs=1) as pp, \
             tc.tile_pool(name="scratch", bufs=2) as sp:
            wbT = pp.tile([128, KT, 2 * H], f32)
            sel3 = pp.tile([2 * H, 3 * H], f32)
            wo = pp.tile([H, OH, H], f32)
            xTr = pp.tile([128, KT, S], f32)
            mask0r = pp.tile([1, S], f32)
            bias2 = pp.tile([2 * H, 1], f32)
            cosT = pp.tile([H, S], f32)
            sinT = pp.tile([H, S], f32)
            tril = pp.tile([128, 896], f32)
            startR = pp.tile([H, S], f32)
            endA = pp.tile([H + 1, S], f32)       # 0..63 endR*pad, 64 addrow
            padB = pp.tile([H, S], f32)
            constband = pp.tile([128, 4, 512], f32)  # 4 copies of const band
            addrow0 = pp.tile([1, S], f32)
            mask0 = pp.tile([1, S], f32)
            tmpA0 = pp.tile([H + 1, S], f32)
            tmpA1 = pp.tile([H + 1, S], f32)

            with tc.tile_pool(name="load", bufs=1) as lp:
                # mask first: it gates the constant-band writes, which should
                # saturate DMA while the rest of prep computes
                nc.sync.dma_start(mask0[:], mask_t.ap())
                ones1f = pp.tile([1, 128], f32)
                nc.gpsimd.memset(ones1f[:], 1.0)
                ones1 = pp.tile([1, 128], f32)
                nc.vector.tensor_copy(r(ones1[:]), ones1f[:])
                onesrow = pp.tile([1, S], f32)
                nc.gpsimd.memset(onesrow[:], 1.0)
                onesrowr = pp.tile([1, S], f32)
                nc.vector.tensor_copy(r(onesrowr[:]), onesrow[:])
                nc.vector.tensor_copy(r(mask0r[:]), mask0[:])
                nc.vector.tensor_scalar(
                    r(addrow0[:]), mask0[:], float(NEG), float(-NEG),
                    ALU.mult, ALU.add)                 # (pad-1)*NEG
                # tiny cross-partition row moves on the scalar HWDGE ring,
                # issued as early as their sources exist: row 64 of endA is
                # addrow, row 64 of each tmpA buffer is ones
                nc.scalar.dma_start(r(endA[H:H + 1, :]), r(addrow0[:]))
                nc.scalar.dma_start(r(tmpA0[H:H + 1, :]), r(onesrowr[:]))
                nc.scalar.dma_start(r(tmpA1[H:H + 1, :]), r(onesrowr[:]))

                # critical-path loads: projection weights + xT chunks
                wbL = lp.tile([128, KT, 2 * H], f32)
                nc.sync.dma_start(
                    wbL[:], wb_t.ap().rearrange("(t p) h -> p t h", p=128))
                nc.vector.tensor_copy(r(wbT[:]), wbL[:])
                selL = lp.tile([2 * H, 3 * H], f32)
                nc.sync.dma_start(selL[:], sel_t.ap())
                nc.scalar.copy(r(sel3[:]), selL[:])
                # xT halves: h=1 first — label 0's first output chunk needs
                # only the h=1 projections. One half-size landing tile is
                # reused for both halves (h=0 lands after h=1 is cast),
                # keeping SBUF free for deep output staging. h=0 casts are
                # emitted after the h=1 rotation so the in-order DVE/ACT
                # queues aren't blocked waiting on h=0 DMAs.
                xTin = lp.tile([128, KT, 512], f32)
                xg = xT_t.ap().rearrange("(t p) s -> p t s", p=128)
                sl1 = slice(512, 1024)
                for t in range(KT):
                    nc.sync.dma_start(xTin[:, t, :], xg[:, t, sl1])
                    if t % 2 == 0:
                        nc.vector.tensor_copy(r(xTr[:, t, sl1]),
                                              xTin[:, t, :])
                    else:
                        nc.scalar.copy(r(xTr[:, t, sl1]), xTin[:, t, :])
                woL = lp.tile([H, OH, H], f32)
                nc.sync.dma_start(woL[:], wo_t.ap().rearrange("o i j -> i o j"))
                nc.scalar.copy(r(wo[:]), woL[:])
                nc.sync.dma_start(cosT[:], cos_t.ap())
                nc.sync.dma_start(sinT[:], sin_t.ap())
                nc.sync.dma_start(bias2[:], b2_t.ap())
                sl0 = slice(0, 512)
                for t in range(KT):
                    nc.sync.dma_start(xTin[:, t, :], xg[:, t, sl0])
                nc.sync.dma_start(tril[:], tril_t.ap())

                with tc.tile_pool(name="psu", bufs=1, space=PSUM) as psu, \
                     tc.tile_pool(name="stg0_pool", bufs=3) as st0, \
                     tc.tile_pool(name="stg1_pool", bufs=3) as st1:

                    def trilpat(k):
                        return tril[:, 384 - 128 * k:896 - 128 * k]

                    def prep_h(h):
                        sl = slice(h * 512, (h + 1) * 512)
                        ps2 = psu.tile([128, 512], f32, name="ps2", tag="big",
                                       bufs=6)
                        for kb in range(KT):
                            nc.tensor.matmul(
                                ps2[:], r(wbT[:, kb, :]), r(xTr[:, kb, sl]),
                                start=(kb == 0), stop=(kb == KT - 1))
                        relu2 = sp.tile([128, 512], f32, name="relu2")
                        nc.scalar.activation(r(relu2[:]), ps2[:], AF.Relu,
                                             bias=bias2[:])
                        swS = psu.tile([H, 512], f32, name="swS", tag="small",
                                       bufs=2)
                        nc.tensor.matmul(swS[:], r(sel3[:, 0:H]), r(relu2[:]),
                                         start=True, stop=True)
                        exE = psu.tile([H, 512], f32, name="exE", tag="small",
                                       bufs=2)
                        nc.tensor.matmul(exE[:], r(sel3[:, H:2 * H]),
                                         r(relu2[:]), start=True, stop=True)
                        rm = sp.tile([H, 512], f32, name="rm")
                        nc.vector.tensor_mul(rm[:], relu2[0:H, :], cosT[:, sl])
                        rs = sp.tile([H, 512], f32, name="rs")
                        nc.vector.tensor_mul(rs[:], swS[:], sinT[:, sl])
                        nc.vector.tensor_add(r(startR[:, sl]), rm[:], rs[:])
                        swE = psu.tile([H, 512], f32, name="swE", tag="small",
                                       bufs=2)
                        nc.tensor.matmul(swE[:], r(sel3[:, 2 * H:3 * H]),
                                         r(relu2[:]), start=True, stop=True)
                        rm2 = sp.tile([H, 512], f32, name="rm2")
                        nc.vector.tensor_mul(rm2[:], exE[:], cosT[:, sl])
                        rs2 = sp.tile([H, 512], f32, name="rs2")
                        nc.vector.tensor_mul(rs2[:], swE[:], sinT[:, sl])
                        es = sp.tile([H, 512], f32, name="es")
                        nc.vector.tensor_add(es[:], rm2[:], rs2[:])
                        nc.vector.tensor_mul(r(endA[0:H, sl]), es[:],
                                             padB[:, sl])

                    def tmp_mm(o, h):
                        tmpA = tmpA0 if o % 2 == 0 else tmpA1
                        sl = slice(h * 512, (h + 1) * 512)
                        ps_tmp = psu.tile([H, 512], f32, name="ps_tmp",
                                          tag="small", bufs=2)
                        nc.tensor.matmul(ps_tmp[:],
                                         r(wo[:, o, :]), r(startR[:, sl]),
                                         start=True, stop=True)
                        nc.scalar.copy(r(tmpA[0:H, sl]), ps_tmp[:])

                    def chunk1(o):
                        tmpA = tmpA0 if o % 2 == 0 else tmpA1
                        stg1 = st1.tile([128, 4, 512], f32, name="stg1")
                        for xb in range(4, 8):
                            lhs = r(tmpA[:, xb * 128:(xb + 1) * 128])
                            ps_sp3 = psu.tile([128, 512], f32, name="ps_sp3",
                                              tag="big", bufs=6)
                            nc.tensor.matmul(ps_sp3[:], lhs,
                                             r(endA[:, 512:1024]),
                                             start=True, stop=True)
                            nc.vector.tensor_tensor(stg1[:, xb - 4, :],
                                                    ps_sp3[:],
                                                    trilpat(xb - 4), ALU.add)
                        nc.sync.dma_start(out_r[o, 1][:, :, 512:1024], stg1[:])

                    def chunk0(o):
                        tmpA = tmpA0 if o % 2 == 0 else tmpA1
                        stg0 = st0.tile([128, 4, S], f32, name="stg0")
                        for xb in range(4):
                            lhs = r(tmpA[:, xb * 128:(xb + 1) * 128])
                            ps_sp = psu.tile([128, 512], f32, name="ps_sp",
                                             tag="big", bufs=6)
                            nc.tensor.matmul(ps_sp[:], lhs, r(endA[:, 0:512]),
                                             start=True, stop=True)
                            nc.vector.tensor_tensor(stg0[:, xb, 0:512],
                                                    ps_sp[:],
                                                    trilpat(xb), ALU.add)
                            ps_sp2 = psu.tile([128, 512], f32, name="ps_sp2",
                                              tag="big", bufs=6)
                            nc.tensor.matmul(ps_sp2[:], lhs,
                                             r(endA[:, 512:1024]),
                                             start=True, stop=True)
                            nc.scalar.copy(stg0[:, xb, 512:1024], ps_sp2[:])
                        nc.sync.dma_start(out_r[o, 0], stg0[:])

                    # pad broadcast + constant band via K=1 fp32r matmuls
                    for h in range(2):
                        sl = slice(h * 512, (h + 1) * 512)
                        ps_pb = psu.tile([H, 512], f32, name="ps_pb",
                                         tag="small", bufs=2)
                        nc.tensor.matmul(ps_pb[:], r(ones1[:, :H]),
                                         r(mask0r[:, sl]),
                                         start=True, stop=True)
                        nc.scalar.copy(padB[:, sl], ps_pb[:])
                    ps_cb = psu.tile([128, 512], f32, name="ps_cb", tag="big",
                                     bufs=6)
                    nc.tensor.matmul(ps_cb[:], r(ones1[:]), r(addrow0[:, 0:512]),
                                     start=True, stop=True)
                    nc.scalar.activation(constband[:, 0, :], ps_cb[:], AF.Copy,
                                         bias=float(-NEG))
                    for j in range(1, 4):
                        nc.scalar.copy(constband[:, j, :], constband[:, 0, :])
                    # constant (below-diagonal) output regions for every label,
                    # on the SWDGE queue: background traffic during prep
                    for o in range(OH):
                        nc.gpsimd.dma_start(out_r[o, 1][:, :, 0:512],
                                            constband[:])

                    # h=1 prep, then label 0 chunk 1 immediately — the PE is
                    # in-order, so emit the first output's matmuls before the
                    # h=0 projection block
                    prep_h(1)
                    tmp_mm(0, 1)
                    chunk1(0)
                    # label 1's chunk1 also needs only h=1 data — fill the
                    # window while prep_h(0) hasn't produced anything yet
                    tmp_mm(1, 1)
                    chunk1(1)
                    # now the h=0 xT casts (their DMAs were issued above)
                    for t in range(KT):
                        if t % 2 == 0:
                            nc.vector.tensor_copy(r(xTr[:, t, sl0]),
                                                  xTin[:, t, :])
                        else:
                            nc.scalar.copy(r(xTr[:, t, sl0]), xTin[:, t, :])
                    prep_h(0)
                    tmp_mm(0, 0)
                    chunk0(0)
                    for o in range(1, OH):
                        if o >= 2:
                            tmp_mm(o, 1)
                            chunk1(o)
                        tmp_mm(o, 0)
                        chunk0(o)

    nc.compile()
    return nc


def _get_nc():
    if "nc" not in _STATE:
        _STATE["nc"] = _build()
    return _STATE["nc"]


def _make_in_maps(x, mask, W_start, b_start, W_end, b_end, weight):
    cosT, sinT, sel, tril = _tables()
    x = np.asarray(x, np.float32)
    mask = np.ascontiguousarray(np.asarray(mask, np.float32))
    W_start = np.asarray(W_start, np.float32)
    W_end = np.asarray(W_end, np.float32)
    w_both = np.ascontiguousarray(np.concatenate([W_start, W_end], axis=1))
    bias2 = np.ascontiguousarray(
        np.concatenate([np.asarray(b_start, np.float32).reshape(H),
                        np.asarray(b_end, np.float32).reshape(H)]).reshape(
                            2 * H, 1))
    weight = np.ascontiguousarray(np.asarray(weight, np.float32))
    in_maps = []
    for c in range(NCORES):
        b, half = c // 2, c % 2
        in_maps.append({
            "xT": np.ascontiguousarray(x[b].T),
            "mask": np.ascontiguousarray(mask[b:b + 1]),
            "w_both": w_both,
            "bias2": bias2,
            "w_o": np.ascontiguousarray(weight[half * OH:(half + 1) * OH]),
            "cos_t": cosT,
            "sin_t": sinT,
            "sel3": sel,
            "trilneg": tril,
        })
    return in_maps


def _execute(in_maps, trace=False):
    from concourse.bass_utils import run_bass_kernel_spmd
    nc = _get_nc()
    return run_bass_kernel_spmd(nc, in_maps, list(range(NCORES)), trace=trace)


def kernel(x, mask, W_start, b_start, W_end, b_end, weight):
    in_maps = _make_in_maps(x, mask, W_start, b_start, W_end, b_end, weight)
    res = _execute(in_maps)
    outs = [res.results[c]["out"] for c in range(NCORES)]
    full = np.stack(outs).reshape(B, 2, OH, S, S).reshape(B, O, S, S)
    return full.astype(np.float32)



# revision 8
# speedup vs baseline: 1.5260x; 1.5260x over previous
"""Biaffine span classifier kernel for 8 Trainium2 NeuronCores.

Math (per batch b, label o):
    start = relu(x @ W_start + b_start); end = relu(x @ W_end + b_end)
    rotate both with tiled-halves sinusoidal tables
    span[o,x,y] = startR[x,:] @ weight[o] @ endR[y,:]^T
    span = span*pad[y] - (1-pad[y])*NEG - NEG*tril(x>y)

Sharding: core c = b*2 + half handles batch b and labels [half*8, half*8+8).

The kernel is HBM-bound, so the device moves as few bytes as possible:
  * The output is written in bf16 (per-element tolerance is 2e-2; fp32r
    matmul + bf16 rounding lands well under 1e-2) and upconverted on the
    host during the gather step.
  * Every entry at or below the diagonal band, and every masked column, is
    a value computable from `mask` alone in exact fp32 (-NEG, -2*NEG, or
    -NEG*(2-pad[y]) -- |span| << 0.5*ulp(NEG) so the reference's own fp32
    adds round to exactly these). The device only computes/writes the 36
    upper-triangular 128x128 blocks per label as eight row bands
    (rows [128k,128k+128) x cols [128k,1024)); the host fills the rest
    during unsharding. Device writes drop from 32 MB to 9.4 MB per core.
  * All matmuls run fp32r (full PE rate at free-dim >= 256). DRAM inputs
    are declared float32r so DMA lands them matmul-ready with no cast
    copies.

PE utilization trick: the span contraction is K=64, which would idle half
the 128-row PE array. startR/endR are produced DUPLICATED across both
partition halves (via selector matmuls whose outputs live on partitions
0-63 and 64-127), and the per-label tmp = W_o^T startR matmuls emit label
pairs (2g, 2g+1) onto rows 0-63 / 64-127 of one tile. Span matmuls for a
label pair then issue as two K=64 matmuls at tile_position (0,0) and
(64,0) -- distinct PE row groups -- so they stream concurrently (~2x).

PSUM->SBUF bf16 casts are split DVE (even label) / ACT (odd label); the
rotation multiplies run on DVE, the adds on GpSimd (SBUF-only). Output
DMAs alternate the sync/scalar HWDGE rings; the second half of x loads on
the gpsimd SWDGE ring so they don't head-of-line-block output writes.
"""

import numpy as np

B, S, I, H, O = 4, 1024, 768, 64, 16
NCORES = 8
OH = O // 2  # 8 labels per core
NEG = 1.0e12
KT = I // 128  # 6 k-tiles over the input dim

# band xb covers rows [128xb, 128xb+128) x cols [128xb, 1024), computed in
# chunks of 256..512 columns (fp32r needs free-dim >= 256 for full rate).
# band 7 computes cols [768,1024) but only casts/writes [896,1024).
BAND_CHUNKS = {
    0: [(0, 512), (512, 1024)],
    1: [(128, 512), (512, 1024)],
    2: [(256, 512), (512, 1024)],
    3: [(384, 768), (768, 1024)],
    4: [(512, 1024)],
    5: [(640, 1024)],
    6: [(768, 1024)],
    7: [(768, 1024)],
}

_STATE = {}


def _tables():
    """Host-precomputed constants (mimic reference fp32 ops)."""
    position = np.arange(S, dtype=np.float32)
    idx = np.arange(H // 2, dtype=np.float32)
    expo = (np.float32(-2.0) * idx) / np.float32(H)
    inv_freq = np.power(np.float32(10000.0), expo).astype(np.float32)
    ang = position[:, None] * inv_freq[None, :]          # [S, 32] f32
    cos_h = np.cos(ang).astype(np.float32).T             # [32, S]
    sin_h = np.sin(ang).astype(np.float32).T
    cos2 = np.ascontiguousarray(np.tile(cos_h, (4, 1)))  # [128, S]
    sin2 = np.ascontiguousarray(np.tile(sin_h, (4, 1)))
    # selector lhsT [128, 512]: 4 column blocks of 128, each mapping the
    # stacked [start;end] projection rows to DUPLICATED outputs (rows 0-63
    # and 64-127 identical). msw: out[2m] = -in[2m+1]; out[2m+1] = in[2m].
    sel = np.zeros((2 * H, 4 * 2 * H), np.float32)
    for d in range(2):  # duplicate halves of the output
        mo = 64 * d
        for j in range(H):
            sel[j, 0 + mo + j] = 1.0               # start dup
            sel[H + j, 256 + mo + j] = 1.0         # end dup
        for m in range(H // 2):
            sel[2 * m + 1, 128 + mo + 2 * m] = -1.0      # start swap
            sel[2 * m, 128 + mo + 2 * m + 1] = 1.0
            sel[H + 2 * m + 1, 384 + mo + 2 * m] = -1.0  # end swap
            sel[H + 2 * m, 384 + mo + 2 * m + 1] = 1.0
    return cos2, sin2, sel


def _build():
    import concourse.bacc as bacc
    import concourse.bass as bass
    import concourse.mybir as mybir
    from concourse import tile

    f32 = mybir.dt.float32
    f32r = mybir.dt.float32r
    bf16 = mybir.dt.bfloat16
    AF = mybir.ActivationFunctionType
    ALU = mybir.AluOpType
    PSUM = bass.MemorySpace.PSUM

    nc = bacc.Bacc("TRN2", target_bir_lowering=False, debug=False,
                   num_devices=NCORES)

    # xTp / wbp are host-preswizzled to [partition, ...] so every input DMA
    # lands with one large contiguous descriptor per partition
    xT_t = nc.dram_tensor("xTp", [128, KT, S], f32r, kind="ExternalInput")
    wb_t = nc.dram_tensor("wbp", [128, KT, 2 * H], f32r,
                          kind="ExternalInput")
    b2_t = nc.dram_tensor("bias2", [2 * H, 1], f32, kind="ExternalInput")
    wo2_t = nc.dram_tensor("wo2", [2 * H, 2, 2 * H], f32r,
                           kind="ExternalInput")
    cos_t = nc.dram_tensor("cos2", [2 * H, S], f32, kind="ExternalInput")
    sin_t = nc.dram_tensor("sin2", [2 * H, S], f32, kind="ExternalInput")
    sel_t = nc.dram_tensor("sel4", [2 * H, 4 * 2 * H], f32r,
                           kind="ExternalInput")
    out_t = nc.dram_tensor("out", [OH, S, S], bf16, kind="ExternalOutput")

    # [o, xb, p, y]: row = 128*xb + p
    out_b = out_t.ap().rearrange("o (xb p) y -> o xb p y", xb=8, p=128)
    xg = xT_t.ap()

    def r(ap):
        return ap.bitcast(f32r)

    with tile.TileContext(nc) as tc:
        with tc.tile_pool(name="persist", bufs=1) as pp, \
             tc.tile_pool(name="scratch", bufs=2) as sp, \
             tc.tile_pool(name="stage", bufs=4) as stp, \
             tc.tile_pool(name="psu", bufs=1, space=PSUM) as psu:

            xTr = pp.tile([128, KT, S], f32r)
            wbT = pp.tile([128, KT, 2 * H], f32r)
            sel4 = pp.tile([2 * H, 4 * 2 * H], f32r)
            wo2 = pp.tile([2 * H, 2, 2 * H], f32r)
            bias2 = pp.tile([2 * H, 1], f32)
            cos2 = pp.tile([2 * H, S], f32)
            sin2 = pp.tile([2 * H, S], f32)
            startR2 = pp.tile([2 * H, S], f32)
            endR2 = pp.tile([2 * H, S], f32)
            tmp2 = pp.tile([2 * H, 4, S], f32)

            sl1 = slice(512, 1024)
            sl0 = slice(0, 512)

            # input loads spread across the three DMA-issuing queues so
            # descriptor generation (~0.6us per dma_start) pipelines:
            # sync gets the critical-path h=1 loads, scalar the small
            # constants, gpsimd(SWDGE) the h=0 half of x
            nc.sync.dma_start(wbT[:], wb_t.ap())
            nc.sync.dma_start(xTr[:, 0:3, sl1], xg[:, 0:3, sl1])
            nc.sync.dma_start(xTr[:, 3:6, sl1], xg[:, 3:6, sl1])
            nc.scalar.dma_start(bias2[:], b2_t.ap())
            nc.scalar.dma_start(sel4[:], sel_t.ap())
            nc.scalar.dma_start(cos2[:], cos_t.ap())
            nc.scalar.dma_start(sin2[:], sin_t.ap())
            nc.scalar.dma_start(wo2[:], wo2_t.ap())
            nc.gpsimd.dma_start(xTr[:, :, sl0], xg[:, :, sl0])

            def prep_h(h):
                sl = slice(h * 512, (h + 1) * 512)
                ps2 = psu.tile([128, 512], f32, name="ps2", tag="t", bufs=2)
                for kb in range(KT):
                    nc.tensor.matmul(ps2[:], wbT[:, kb, :], xTr[:, kb, sl],
                                     start=(kb == 0), stop=(kb == KT - 1))
                relu2 = sp.tile([128, 512], f32, name="relu2")
                nc.scalar.activation(r(relu2[:]), ps2[:], AF.Relu,
                                     bias=bias2[:])
                ps_sd = psu.tile([128, 512], f32, name="ps_sd", tag="a",
                                 bufs=3)
                nc.tensor.matmul(ps_sd[:], sel4[:, 0:128], r(relu2[:]),
                                 start=True, stop=True)
                ps_sw = psu.tile([128, 512], f32, name="ps_sw", tag="b",
                                 bufs=3)
                nc.tensor.matmul(ps_sw[:], sel4[:, 128:256], r(relu2[:]),
                                 start=True, stop=True)
                t_sd = sp.tile([128, 512], f32, name="t_sd")
                nc.vector.tensor_mul(t_sd[:], ps_sd[:], cos2[:, sl])
                t_sw = sp.tile([128, 512], f32, name="t_sw")
                nc.vector.tensor_mul(t_sw[:], ps_sw[:], sin2[:, sl])
                nc.gpsimd.tensor_tensor(r(startR2[:, sl]), t_sd[:], t_sw[:],
                                        ALU.add)
                ps_ed = psu.tile([128, 512], f32, name="ps_ed", tag="a",
                                 bufs=3)
                nc.tensor.matmul(ps_ed[:], sel4[:, 256:384], r(relu2[:]),
                                 start=True, stop=True)
                ps_ew = psu.tile([128, 512], f32, name="ps_ew", tag="b",
                                 bufs=3)
                nc.tensor.matmul(ps_ew[:], sel4[:, 384:512], r(relu2[:]),
                                 start=True, stop=True)
                t_ed = sp.tile([128, 512], f32, name="t_ed")
                nc.vector.tensor_mul(t_ed[:], ps_ed[:], cos2[:, sl])
                t_ew = sp.tile([128, 512], f32, name="t_ew")
                nc.vector.tensor_mul(t_ew[:], ps_ew[:], sin2[:, sl])
                nc.gpsimd.tensor_tensor(r(endR2[:, sl]), t_ed[:], t_ew[:],
                                        ALU.add)

            def tmp_g(g, h):
                # tmp for labels (2g, 2g+1) on rows 0-63 / 64-127; groups
                # with ph=0/1 use distinct PE row groups -> concurrent
                pg, ph = g // 2, g % 2
                sl = slice(h * 512, (h + 1) * 512)
                ps_t = psu.tile([128, 512], f32, name=f"ps_t{ph}", tag="t",
                                bufs=2)
                nc.tensor.matmul(ps_t[:], wo2[64 * ph:64 * ph + 64, pg, :],
                                 r(startR2[64 * ph:64 * ph + 64, sl]),
                                 start=True, stop=True,
                                 tile_position=(64 * ph, 0))
                if ph == 0:
                    nc.vector.tensor_copy(r(tmp2[:, g, sl]), ps_t[:])
                else:
                    nc.scalar.copy(r(tmp2[:, g, sl]), ps_t[:])

            def band(g, xb):
                w0 = 128 * xb
                wb = 1024 - w0
                stA = stp.tile([128, 1024], bf16, name="stA")
                stB = stp.tile([128, 1024], bf16, name="stB")
                for (c0, c1) in BAND_CHUNKS[xb]:
                    n = c1 - c0
                    lhsA = r(tmp2[0:64, g, w0:w0 + 128])
                    psA = psu.tile([128, 512], f32, name="psA", tag="a",
                                   bufs=3)
                    nc.tensor.matmul(psA[:, 0:n], lhsA,
                                     r(endR2[0:64, c0:c1]),
                                     start=True, stop=True,
                                     tile_position=(0, 0))
                    lhsB = r(tmp2[64:128, g, w0:w0 + 128])
                    psB = psu.tile([128, 512], f32, name="psB", tag="b",
                                   bufs=3)
                    nc.tensor.matmul(psB[:, 0:n], lhsB,
                                     r(endR2[64:128, c0:c1]),
                                     start=True, stop=True,
                                     tile_position=(64, 0))
                    d0 = max(c0, w0) - w0
                    s0 = max(0, w0 - c0)
                    nc.vector.tensor_copy(stA[:, d0:c1 - w0],
                                          psA[:, s0:n])
                    nc.scalar.copy(stB[:, d0:c1 - w0], psB[:, s0:n])
                nc.sync.dma_start(out_b[2 * g, xb][:, w0:1024],
                                  stA[:, 0:wb])
                nc.gpsimd.dma_start(out_b[2 * g + 1, xb][:, w0:1024],
                                    stB[:, 0:wb])

            # all prep first, then one uniform 32-band pipeline with
            # consecutive per-engine instructions independent (g varies
            # fastest) so semaphore latencies overlap instead of chaining
            prep_h(1)
            for g in range(4):
                tmp_g(g, 1)
            prep_h(0)
            for g in range(4):
                tmp_g(g, 0)
            for xb in (4, 5, 6, 7, 0, 1, 2, 3):
                for g in range(4):
                    band(g, xb)

    nc.compile()
    return nc


def _get_nc():
    if "nc" not in _STATE:
        _STATE["nc"] = _build()
    return _STATE["nc"]


def _make_in_maps(x, mask, W_start, b_start, W_end, b_end, weight):
    cos2, sin2, sel = _tables()
    x = np.asarray(x, np.float32)
    W_start = np.asarray(W_start, np.float32)
    W_end = np.asarray(W_end, np.float32)
    w_both = np.ascontiguousarray(np.concatenate([W_start, W_end], axis=1))
    bias2 = np.ascontiguousarray(
        np.concatenate([np.asarray(b_start, np.float32).reshape(H),
                        np.asarray(b_end, np.float32).reshape(H)]).reshape(
                            2 * H, 1))
    weight = np.asarray(weight, np.float32)
    # pre-swizzle to [partition, t, ...] so DMA descriptors are one large
    # contiguous run per partition
    xTs = [np.ascontiguousarray(
        x[b].T.reshape(KT, 128, S).transpose(1, 0, 2)) for b in range(B)]
    wbp = np.ascontiguousarray(
        w_both.reshape(KT, 128, 2 * H).transpose(1, 0, 2))
    wo2s = []
    for half in range(2):
        wl = weight[half * OH:(half + 1) * OH]
        wo2 = np.zeros((2 * H, 2, 2 * H), np.float32)
        for pg in range(2):
            for ph in range(2):
                for u in range(2):
                    o = 2 * (2 * pg + ph) + u
                    wo2[64 * ph:64 * ph + 64, pg, 64 * u:64 * u + 64] = wl[o]
        wo2s.append(np.ascontiguousarray(wo2))
    in_maps = []
    for c in range(NCORES):
        b, half = c // 2, c % 2
        in_maps.append({
            "xTp": xTs[b],
            "wbp": wbp,
            "bias2": bias2,
            "wo2": wo2s[half],
            "cos2": cos2,
            "sin2": sin2,
            "sel4": sel,
        })
    return in_maps


def _assemble(outs, mask):
    """Gather per-core band outputs into the full fp32 result, filling the
    mask-determined entries (masked columns, below-diagonal region) with
    their exact fp32 values."""
    mask = np.asarray(mask, np.float32)
    full = np.empty((B, O, S, S), np.float32)
    for c in range(NCORES):
        b, half = c // 2, c % 2
        full[b, half * OH:(half + 1) * OH] = \
            np.asarray(outs[c]).astype(np.float32)
    tri = np.tri(S, S, -1, dtype=bool)  # [x, y]: x > y
    for b in range(B):
        pad = mask[b]
        cols0 = np.nonzero(pad == 0.0)[0]
        if cols0.size:
            full[b][:, :, cols0] = np.float32(-NEG)
        below = (np.float32(-NEG) * (np.float32(2.0) - pad)).astype(
            np.float32)                                   # [y]
        full[b][:, tri] = np.broadcast_to(below, (S, S))[tri]
    return full


def _execute(in_maps, trace=False):
    from concourse.bass_utils import run_bass_kernel_spmd
    nc = _get_nc()
    return run_bass_kernel_spmd(nc, in_maps, list(range(NCORES)), trace=trace)


def kernel(x, mask, W_start, b_start, W_end, b_end, weight):
    in_maps = _make_in_maps(x, mask, W_start, b_start, W_end, b_end, weight)
    res = _execute(in_maps)
    outs = [res.results[c]["out"] for c in range(NCORES)]
    return _assemble(outs, mask)
